# revision 98
# baseline (speedup 1.0000x reference)
"""GQA attention block (RoPE + causal attention + out-proj) on 8 TRN2 cores.

Problem: nn_AdvancedAttn (B=2, S=2048, DIM=2048, H=16 q-heads, KVH=4 kv-heads,
DH=128), fp32 in/out.

Sharding: core (b, g) for b in {0,1}, g in {0..3} handles batch b and kv-head
group g (4 query heads + 1 kv head).  Wq/Wk/Wv are split along the head dim,
Wo along its input dim; the 4 partial Wo outputs per batch are summed on host
(the all-reduce of tensor parallelism).

Device kernel (per core):
  - The Q/K/V and output projections run as fp8 DoubleRow matmuls (0.5
    cycles/row) on a 3-product hi/lo expansion: each operand is split into
    two e4m3 codes (hi = e4m3(t), lo = e4m3(t - hi)) and the product
    (hi+lo)x(hi+lo) is computed dropping the lo*lo term.  This costs 0.75
    cycles/row instead of fp32r's 1.0 with ~tf32 accuracy.  Chunk pairs ride
    the two DoubleRow k-tiles, so no operand duplication is needed.  x / W
    codes are prepared on host; the attention-output codes are produced on
    device during softmax normalization.
  - Scores stay fp32r with Q^T/K^T produced in [dh, s] layout (RoPE fused
    into the PSUM eviction; the hi/lo projection descale rides the host
    cos/sin tables).  Scores are computed transposed so exp tiles feed A@V
    directly; exp tiles are bf16.
  - Row-sums use the stationary-operand trick: the exp tile is the loaded
    (stationary) matmul operand and a 1-wide bf16 ones vector is the moving
    operand, so each row-sum matmul streams a single row instead of the full
    512-wide tile.  PSUM allows only one open accumulation group per bank,
    so each (chunk, query-slice) partial commits into its own column of a
    [128, 4, 16] scratch bank (ragged widths padded by zero outer products)
    and a DVE add-tree reduces the columns; the reciprocal is taken in
    [query, slice] orientation and a transposing DMA restores the [1, 512]
    layout for the partition-broadcast used by normalization.  The final
    tile instead reduces per-slice on ACT (accum_out) and rebuilds the
    broadcast on-chip via PE transpose + ones outer product, so the tail
    out-projection never waits on a DRAM roundtrip.
  - Causal masking: additive bf16 mask blocks ride the PE as an identity
    matmul accumulating onto the scores, keeping the per-block chain
    PE -> ACT(exp) -> PE with no cross-engine queue parking; fully-masked
    score blocks are skipped entirely, and per-block leading masked columns
    narrow the compute window (capped so fp32r matmuls stay >=256 wide).
  - Engine-queue discipline drives the schedule: x-tile DMAs issue on the
    SP/SWDGE queues with a buffer per s-tile (a parked dma_start blocks its
    issuing sequencer), the broadcast DMAs fire a tile ahead of their
    consumer, and the last s-tile's outputs are permuted across PSUM slots
    so each bank's release precedes its first attention use.
  - y is stored fp16 (the partial sums are small); host accumulates in fp32.
"""
import json
import math

import ml_dtypes
import numpy as np

import concourse.bass as bass
import concourse.mybir as mybir
import concourse.tile as tile
from concourse.bass_utils import run_bass_kernel_spmd

# ---------------------------------------------------------------- constants
B = 2
S = 2048
DIM = 2048
H = 16
KVH = 4
DH = 128
HPC = 4           # query heads per core
NCORE = 8
THETA = 10000.0
P = 128
ST = 512          # s-tile width (sequence) for projections / attention rhs
NCH = DIM // P    # 16 contraction chunks
NPAIR = NCH // 2  # 8 DoubleRow chunk pairs
NST = S // ST     # 4 s-tiles
SCALE = 1.0 / math.sqrt(DH)
F32 = mybir.dt.float32
F32R = mybir.dt.float32r
F16 = mybir.dt.float16
BF16 = mybir.dt.bfloat16
F8 = mybir.dt.float8e4
E4NP = ml_dtypes.float8_e4m3
DR = mybir.MatmulPerfMode.DoubleRow
NEG_THRESH = -1e30
OC = 4            # 2^OC scale on device-encoded attention-out codes
MAX_RESIDENT_MIXED = 24

# ------------------------------------------------- walrus multi-wait fixup
# This toolchain's walrus supports fewer sync-waits per instruction than
# Tile emits (observed: Matmult chokes at 2, Drain at 3).  Splitting excess
# waits onto NoOps on the same engine queue immediately before the
# instruction is semantically identical (the engine stalls at the NoOp).
_SKIP_OPCODES = {"CollectiveCompute"}


def _split_waits_in_bir(d: dict) -> None:
    for fn in d.get("functions", []):
        for blk in fn.get("blocks", []):
            out = []
            for inst in blk.get("instructions", []):
                si = inst.get("sync_info")
                waits = (si or {}).get("on_wait") or []
                if len(waits) > 1 and inst.get("opcode", "") not in _SKIP_OPCODES:
                    for k, w in enumerate(waits[1:]):
                        out.append({
                            "debug": inst.get("debug", 0),
                            "engine": inst["engine"],
                            "ins": [],
                            "name": f"{inst['name']}-wsplit{k}",
                            "opcode": "NoOp",
                            "outs": [],
                            "sync_info": {"on_update": [], "on_wait": [w]},
                        })
                    si["on_wait"] = waits[:1]
                out.append(inst)
            blk["instructions"] = out


_waitfix_installed = False


def _install_waitfix():
    global _waitfix_installed
    if _waitfix_installed:
        return
    orig = bass.Bass.to_json_bytes

    def to_json_bytes_split(self):
        d = json.loads(orig(self))
        _split_waits_in_bir(d)
        return json.dumps(d).encode()

    bass.Bass.to_json_bytes = to_json_bytes_split
    _waitfix_installed = True


# ------------------------------------------------------------ program build
def build_program(block_kind, scales, mask_exact_binary=True):
    """block_kind[(c, t)]: ('allow', j0) | 'skip' | ('mixed', idx, j0, j0a, j1a)
    for sk-chunk c x sq-tile t of the transposed mask; j0 = PE compute-window
    start (capped 256), [j0a, j1a) = mask window.
    scales: dict with log2 exponents ax/bqk/bv/bo (host code scales)."""
    _install_waitfix()
    from contextlib import ExitStack
    n_mixed = sum(1 for v in block_kind.values()
                  if isinstance(v, tuple) and v[0] == "mixed")
    resident = n_mixed <= MAX_RESIDENT_MIXED
    mdt = BF16 if mask_exact_binary else F32R
    chunks_of = {}
    for t in range(NST):
        ch = [(c,) + ((0,) if block_kind[(c, t)][0] == "allow"
                      else block_kind[(c, t)][2:])
              for c in range(NCH) if block_kind[(c, t)] != "skip"]
        # first processed chunk must cover the full window so the PSUM
        # accumulation group starts every column
        if ch:
            ch[0] = (ch[0][0], 0) + ch[0][2:]
        chunks_of[t] = ch
    NSL = ST // P  # 4 query slices per s-tile (rowsum granularity)

    v_desc = 2.0 ** (-(scales["ax"] + scales["bv"]))
    y_desc = 2.0 ** (-(OC + scales["bo"]))

    wmax = 1
    for v in block_kind.values():
        if isinstance(v, tuple) and v[0] == "mixed":
            wmax = max(wmax, v[4] - v[3])

    nc = bass.Bass("TRN2", target_bir_lowering=False, debug=False)
    # x codes stay in the pair-interleaved [dim, s] layout (512B runs)
    xh = nc.declare_dram_parameter("xh", [DIM, S], F8, isOutput=False)
    xl = nc.declare_dram_parameter("xl", [DIM, S], F8, isOutput=False)
    # packed projection weights, host-arranged in exact SBUF tile layout:
    # wvk h/l: [P, NPAIR, 2, 256] with v at cols 0:128, k at 128:256
    # (separate hi/lo DMAs so the first x tile can slot between them on the
    # serial DMA device); wq: hi/lo merged, needed later
    wvkh = nc.declare_dram_parameter("wvkh", [P, NPAIR, 2, 2 * DH], F8,
                                     isOutput=False)
    wvkl = nc.declare_dram_parameter("wvkl", [P, NPAIR, 2, 2 * DH], F8,
                                     isOutput=False)
    wqhl = nc.declare_dram_parameter("wqhl", [P, 2, NPAIR, 2, HPC * DH], F8,
                                     isOutput=False)
    woh = nc.declare_dram_parameter("woh", [HPC * DH, DIM], F8, isOutput=False)
    wol = nc.declare_dram_parameter("wol", [HPC * DH, DIM], F8, isOutput=False)
    cstab = nc.declare_dram_parameter(
        "cstab", [DH, NST, 2, ST], F32, isOutput=False)
    onescol = nc.declare_dram_parameter("onescol", [P, 1], BF16, isOutput=False)
    zerocol = nc.declare_dram_parameter("zerocol", [P, 1], BF16, isOutput=False)
    onesrow = nc.declare_dram_parameter("onesrow", [1, P], F32R, isOutput=False)
    ident = nc.declare_dram_parameter("ident", [P, P], F32R, isOutput=False)
    identb = nc.declare_dram_parameter("identb", [P, P], BF16, isOutput=False)
    if n_mixed:
        mmask = nc.declare_dram_parameter(
            "mmask", [n_mixed, P, wmax], mdt, isOutput=False)
    y = nc.declare_dram_parameter("y", [S, DIM], F16, isOutput=True)
    rs_scr = nc.dram_tensor("rs_scr", [HPC * NST, 1, ST], F32)

    allp = mybir.AluOpType
    AF = mybir.ActivationFunctionType

    # pair-interleaved views: chunk pair j rows = j*256 + two*128 + p
    def pair_view(t3):
        return t3[:].rearrange("(j two p) m -> p j two m", two=2, p=P)

    with tile.TileContext(nc) as tc, ExitStack() as ctx:
        # ---- persistent pool -----------------------------------------
        keep = ctx.enter_context(tc.tile_pool(name="keep", bufs=1))
        ones_c = keep.tile([P, 1], BF16)
        zero_c = keep.tile([P, 1], BF16)
        ones_r = keep.tile([1, P], F32R)
        # per-head Q^T and per-s-tile K^T / V tiles: attention reads then
        # depend only on the producing s-tile's eviction, not the last one
        qTs = [keep.tile([P, S], F32R, name=f"qT{h}") for h in range(HPC)]
        kTs = [keep.tile([P, ST], F32R, name=f"kT{t}") for t in range(NST)]
        vsbs = [keep.tile([P, ST // P, DH], BF16, name=f"vsb{t}")
                for t in range(NST)]
        oh = keep.tile([P, HPC, S], F8)      # attention out^T hi code
        ol = keep.tile([P, HPC, S], F8)      # attention out^T lo code
        woh_sb = keep.tile([P, HPC, DIM], F8)
        wol_sb = keep.tile([P, HPC, DIM], F8)
        id_sb = keep.tile([P, P], F32R)
        id_bf = keep.tile([P, P], BF16)
        if n_mixed and resident:
            mm_sb = keep.tile([P, n_mixed, wmax], mdt)

        # ---- projection phase ----------------------------------------
        with ExitStack() as pctx:
            wpool = pctx.enter_context(tc.tile_pool(name="wpool", bufs=1))
            wvk_h = wpool.tile([P, NPAIR, 2, 2 * DH], F8)
            wvk_l = wpool.tile([P, NPAIR, 2, 2 * DH], F8)
            wq = wpool.tile([P, 2, NPAIR, 2, HPC * DH], F8)
            # 4 buffers: every s-tile gets its own slot, so no x DMA ever
            # parks its issuing sequencer on a buffer-reuse wait
            xpool = pctx.enter_context(tc.tile_pool(name="xpool", bufs=4))
            rpool = pctx.enter_context(tc.tile_pool(name="rope", bufs=4))
            tabp = pctx.enter_context(tc.tile_pool(name="tabs", bufs=2))
            vT = wpool.tile([P, S], BF16)    # V^T staging (2B for DMA transpose)

            xh3 = pair_view(xh)
            xl3 = pair_view(xl)
            xtiles = {}

            def load_x(st, split=1):
                cols = bass.ts(st, ST)
                xth = xpool.tile([P, NPAIR, 2, ST], F8, tag="xh", name="xth")
                xtl = xpool.tile([P, NPAIR, 2, ST], F8, tag="xl", name="xtl")
                q = NPAIR // split
                for i in range(split):
                    js = slice(i * q, (i + 1) * q)
                    nc.gpsimd.dma_start(
                        out=xth[:, js], in_=xh3[:, js, :, cols])
                    # sync queue: an SP park costs nothing, an ACT park
                    # would block the projection evictions
                    nc.sync.dma_start(
                        out=xtl[:, js], in_=xl3[:, js, :, cols])
                xtiles[st] = (xth, xtl)

            # startup: wvk_h first (first matmul dep), x0 pieces next on
            # gpsimd(SWDGE)/scalar so they interleave with wvk_l on the
            # serial DMA device; wq (needed ~60% into st0) trails on scalar
            nc.sync.dma_start(out=wvk_h[:], in_=wvkh[:])
            load_x(0, split=4)
            nc.sync.dma_start(out=wvk_l[:], in_=wvkl[:])
            nc.sync.dma_start(out=wq[:], in_=wqhl[:])
            nc.sync.dma_start(out=ones_c[:], in_=onescol[:])
            nc.sync.dma_start(out=zero_c[:], in_=zerocol[:])
            nc.sync.dma_start(out=ones_r[:], in_=onesrow[:])
            nc.sync.dma_start(out=id_sb[:], in_=ident[:])
            nc.sync.dma_start(out=id_bf[:], in_=identb[:])
            load_x(1)

            # out-proj weights: needed only once the first s-tile's four
            # heads finish attention; drained one DMA per produce() there
            woh4 = woh[:].rearrange("(h p) n -> p h n", p=P)
            wol4 = wol[:].rearrange("(h p) n -> p h n", p=P)
            deferred = []
            for hh in range(HPC):
                deferred.append((woh_sb[:, hh], woh4[:, hh]))
                deferred.append((wol_sb[:, hh], wol4[:, hh]))

            def rope_evict(dst, ps, cos_t, sin_t, rot_pool=False):
                # dst = ps * cos + rotate_half(ps) * sin  (sign + hi/lo code
                # descale baked into the host tables).  Rotation via two
                # partition-shifted copies straight out of PSUM; cos-mult
                # reads PSUM aligned on DVE.  The last s-tile's rotations go
                # to Pool so the first attention exps aren't queued behind
                # them on ACT.
                srot = rpool.tile([P, ST], F32, tag="srot", name="srot")
                nc.scalar.activation(
                    out=srot[0:64, :], in_=ps[64:128, :], func=AF.Copy)
                nc.scalar.activation(
                    out=srot[64:128, :], in_=ps[0:64, :], func=AF.Copy)
                nc.vector.tensor_tensor(
                    out=srot[:], in0=srot[:], in1=sin_t[:], op=allp.mult)
                with nc.allow_low_precision(reason="f32r 32-bit storage"):
                    nc.vector.tensor_tensor(
                        out=dst, in0=ps[:], in1=cos_t[:], op=allp.mult)
                    nc.vector.tensor_tensor(
                        out=dst, in0=dst, in1=srot[:], op=allp.add)

            pp = pctx.enter_context(
                tc.tile_pool(name="pp", bufs=6, space="PSUM"))
            pt = pctx.enter_context(
                tc.tile_pool(name="pt", bufs=2, space="PSUM"))
            for st in range(NST):
                cols = bass.ts(st, ST)
                cs_t = tabp.tile([P, 2, ST], F32, tag="cs", name="cs_t")
                nc.sync.dma_start(out=cs_t[:], in_=cstab[:, st])
                cos_t = cs_t[:, 0]
                sin_t = cs_t[:, 1]
                ps_list = [pp.tile([P, ST], F32, tag="proj", name=f"proj{j}")
                           for j in range(6)]
                xth, xtl = xtiles.pop(st)
                if st + 2 < NST:
                    load_x(st + 2)
                if st == 1 and n_mixed and resident:
                    # mask blocks: needed by the first attention produce
                    # (sync queue: ACT-queue DMAs couple their completions
                    # into ACT compute waits)
                    nc.sync.dma_start(
                        out=mm_sb[:],
                        in_=mmask[:].rearrange("n p m -> p n m"))

                # output-major: v, k first (ps_list[0:2]) so their PSUM
                # banks recycle early; the attention pools reuse these banks
                # at the phase transition.  At the last s-tile the outputs
                # are permuted across PSUM slots so each bank's release time
                # precedes its first attention use: attention first touches
                # pc(banks 0-2), then po(3,4) and pr(5); bank 5 is released
                # first (q3 processed first) and bank 4 last (v, whose
                # eviction is a single fast ACT copy).
                def wget(w_h, w_l, m0, m1, code, j):
                    if w_h is wq:
                        return wq[:, code, j, :, m0:m1]
                    return (w_h if code == 0 else w_l)[:, j, :, m0:m1]

                VK = (wvk_h, wvk_l)
                if st == NST - 1:
                    # (kind, psum slot): q3 first (bank 5 = pr, needed
                    # earliest in attention), v LAST on bank 4 (po buf1):
                    # its eviction is a single fast ACT copy, so the bank
                    # frees quickest after the final matmuls
                    order = [(5, ps_list[5], (wq, wq), (3 * DH, 4 * DH)),
                             (2, ps_list[0], (wq, wq), (0, DH)),
                             (3, ps_list[1], (wq, wq), (DH, 2 * DH)),
                             (4, ps_list[2], (wq, wq), (2 * DH, 3 * DH)),
                             (1, ps_list[3], VK, (DH, 2 * DH)),
                             (0, ps_list[4], VK, (0, DH))]
                else:
                    order = (
                        [(0, ps_list[0], VK, (0, DH)),
                         (1, ps_list[1], VK, (DH, 2 * DH))]
                        + [(2 + hh, ps_list[2 + hh], (wq, wq),
                            (hh * DH, (hh + 1) * DH)) for hh in range(HPC)])
                rot_pool = st == NST - 1
                if st == 0:
                    # j-major over v/k at startup: the first x quarter feeds
                    # both wvk outputs while later quarters and the larger
                    # wq transfer stream in; q heads then run output-major
                    for j in range(NPAIR):
                        for oi, ps, (w_h, w_l), (m0, m1) in order[:2]:
                            nc.tensor.matmul(
                                ps[:], wget(w_h, w_l, m0, m1, 0, j),
                                xth[:, j], start=j == 0, stop=False,
                                perf_mode=DR)
                            nc.tensor.matmul(
                                ps[:], wget(w_h, w_l, m0, m1, 1, j),
                                xth[:, j], start=False, stop=False,
                                perf_mode=DR)
                            nc.tensor.matmul(
                                ps[:], wget(w_h, w_l, m0, m1, 0, j),
                                xtl[:, j], start=False, stop=j == NPAIR - 1,
                                perf_mode=DR)
                for oi, ps, (w_h, w_l), (m0, m1) in order:
                    if st > 0 or oi >= 2:
                        for j in range(NPAIR):
                            nc.tensor.matmul(
                                ps[:], wget(w_h, w_l, m0, m1, 0, j),
                                xth[:, j], start=j == 0, stop=False,
                                perf_mode=DR)
                            nc.tensor.matmul(
                                ps[:], wget(w_h, w_l, m0, m1, 1, j),
                                xth[:, j], start=False, stop=False,
                                perf_mode=DR)
                            nc.tensor.matmul(
                                ps[:], wget(w_h, w_l, m0, m1, 0, j),
                                xtl[:, j], start=False, stop=j == NPAIR - 1,
                                perf_mode=DR)
                    def v_transposes():
                        # V^T -> V chunks via a single DMA transpose
                        # (out[p, cc, d] = vT[d, cc*128+p]), freeing PE and
                        # ACT of the per-chunk transpose/copy chains
                        nc.sync.dma_start_transpose(
                            out=vsbs[st][:],
                            in_=vT[:, st * ST:(st + 1) * ST])

                    if oi == 0:
                        with nc.allow_low_precision(reason="bf16 V"):
                            nc.scalar.activation(
                                out=vT[:, cols], in_=ps[:], func=AF.Copy,
                                scale=v_desc)
                        if st == NST - 1:
                            v_transposes()
                    elif oi == 1:
                        rope_evict(kTs[st][:, :], ps[:], cos_t, sin_t,
                                   rot_pool)
                        if st != NST - 1:
                            v_transposes()
                    else:
                        rope_evict(qTs[oi - 2][:, cols], ps[:], cos_t, sin_t,
                                   rot_pool)

        # ---- attention + out-projection, interleaved per s-tile ------
        with ExitStack() as actx:
            epool = actx.enter_context(tc.tile_pool(name="epool", bufs=10))
            tpool = actx.enter_context(tc.tile_pool(name="tpool", bufs=6))
            rsp = actx.enter_context(tc.tile_pool(name="rsp", bufs=6))
            ypool = actx.enter_context(tc.tile_pool(name="ypool", bufs=8))
            if n_mixed and not resident:
                mstr = actx.enter_context(tc.tile_pool(name="mstr", bufs=4))
            pc = actx.enter_context(
                tc.tile_pool(name="pc", bufs=3, space="PSUM"))
            po = actx.enter_context(
                tc.tile_pool(name="po", bufs=2, space="PSUM"))
            pr = actx.enter_context(
                tc.tile_pool(name="pr", bufs=1, space="PSUM"))
            py = actx.enter_context(
                tc.tile_pool(name="py", bufs=2, space="PSUM"))

            tiles = []
            for t in range(NST):
                for hh in range(HPC):
                    ch = chunks_of[t]
                    # per query-slice s: last touching chunk, per-chunk
                    # scratch column (PSUM allows only one open accumulation
                    # group per bank, so each (chunk, slice) partial commits
                    # into its own column and ACT accumulates them)
                    sl_last = [None] * NSL
                    colof = {}
                    ccount = [0] * NSL
                    for ci, info in enumerate(ch):
                        j0 = info[1]
                        for s in range(NSL):
                            if j0 < (s + 1) * P:
                                sl_last[s] = ci
                                colof[(ci, s)] = ccount[s]
                                ccount[s] += 1
                    tiles.append({"hh": hh, "t": t, "chunks": ch,
                                  "sl_last": sl_last, "colof": colof,
                                  "ccount": ccount,
                                  "cmax": max(ccount)})
            flat = [(ti, ci) for ti, td in enumerate(tiles)
                    for ci in range(len(td["chunks"]))]
            etiles = {}
            LOOKAHEAD = 4

            def mask_slice(c, t, j0a, j1a):
                idx = block_kind[(c, t)][1]
                if resident:
                    return mm_sb[:, idx, 0:j1a - j0a]
                mtile = mstr.tile([P, wmax], mdt, tag="ms", name="mtile")
                nc.sync.dma_start(out=mtile[:], in_=mmask[idx])
                return mtile[:, 0:j1a - j0a]

            def produce(ti, ci):
                if deferred:  # out-proj weight loads ride early attention
                    dst, src = deferred.pop(0)
                    nc.sync.dma_start(out=dst, in_=src)
                td = tiles[ti]
                info = td["chunks"][ci]
                c, j0 = info[0], info[1]
                t = td["t"]
                cs = slice(t * ST + j0, (t + 1) * ST)
                ps_c = pc.tile([P, ST], F32, tag="c", name="ps_c")
                nc.tensor.matmul(
                    ps_c[:, j0:], kTs[c // NSL][:, (c % NSL) * P:
                                                 (c % NSL + 1) * P],
                    qTs[td["hh"]][:, cs], start=True, stop=True)
                if len(info) == 4:
                    # mask-add rides PE as an identity-matmul accumulation:
                    # keeps the block chain PE -> ACT -> PE with no
                    # cross-engine queue parking
                    _, _, j0a, j1a = info
                    midm = id_bf if mdt == BF16 else id_sb
                    nc.tensor.matmul(
                        ps_c[:, j0a:j1a], midm[:],
                        mask_slice(c, t, j0a, j1a),
                        start=False, stop=True, skip_group_check=True)
                et = epool.tile([P, ST], BF16, tag="e", name="et")
                with nc.allow_low_precision(reason="bf16 attn weights"):
                    nc.scalar.activation(
                        out=et[:, j0:], in_=ps_c[:, j0:], func=AF.Exp,
                        scale=SCALE)
                etiles[(ti, ci)] = et

            last_ti = len(tiles) - 1

            def norm_slices(td, ti, s0, s1):
                # per-slice-group norm chain for the final tile, built from
                # PE transpose + outer-product broadcast (no DMA roundtrip:
                # the HWDGE/DGE latency would sit on the critical tail)
                hh = td["hh"]
                t = td["t"]
                w = s1 - s0 + 1
                rs = rsp.tile([P, NSL], F32R, tag="rs1", name="rs1")
                with nc.allow_low_precision(reason="f32r storage"):
                    nc.vector.reciprocal(
                        out=rs[:, 0:w], in_=td["dsum"][:, s0:s1 + 1])
                # transpose each slice's recip onto partition 0, one copy to
                # SBUF, then ones-row outer products rebuild the broadcast
                ps_tr = pc.tile([1, ST], F32R, tag="c", name="ps_tr")
                for ls in range(w):
                    nc.tensor.transpose(
                        ps_tr[0:1, ls * P:(ls + 1) * P],
                        rs[:, ls:ls + 1], id_sb[:])
                rsT = rsp.tile([1, ST], F32R, tag="rsT", name="rsT")
                with nc.allow_low_precision(reason="f32r storage"):
                    nc.vector.tensor_copy(rsT[0:1, 0:w * P],
                                          ps_tr[0:1, 0:w * P])
                bc_ps = pc.tile([P, ST], F32, tag="c", name="bc_ps")
                for ls in range(w):
                    nc.tensor.matmul(
                        bc_ps[:, ls * P:(ls + 1) * P], ones_r[:],
                        rsT[0:1, ls * P:(ls + 1) * P], start=True, stop=True)
                # stage to SBUF: engines may read only one PSUM operand
                bc_sb = tpool.tile([P, ST], F32, tag="bc1", name="bc_sb")
                nc.vector.tensor_copy(bc_sb[:, 0:w * P], bc_ps[:, 0:w * P])
                cols = slice(t * ST + s0 * P, t * ST + (s1 + 1) * P)
                lc = slice(s0 * P, (s1 + 1) * P)
                tmp = tpool.tile([P, ST], F32, tag="tmp1", name="tmp1")
                nc.vector.tensor_tensor(
                    out=tmp[:, 0:w * P], in0=td["ps_o"][:, lc],
                    in1=bc_sb[:, 0:w * P], op=allp.mult)
                with nc.allow_low_precision(reason="fp8 codes"):
                    nc.vector.tensor_copy(oh[:, hh, cols], tmp[:, 0:w * P])
                    nc.vector.tensor_tensor(
                        out=ol[:, hh, cols], in0=tmp[:, 0:w * P],
                        in1=oh[:, hh, cols], op=allp.subtract)

            def consume(ti, ci):
                td = tiles[ti]
                info = td["chunks"][ci]
                c, j0 = info[0], info[1]
                hh = td["hh"]
                if ci == 0:
                    td["ps_o"] = po.tile([P, ST], F32, tag="o", name="ps_o")
                    td["ps_d"] = pr.tile([P, NSL, NCH], F32, tag="r",
                                         name="ps_d")
                    td["dsum"] = rsp.tile([P, NSL], F32, tag="ds",
                                          name="dsum")
                    td["scr"] = rsp.tile([P, NSL, NCH], F32, tag="sc",
                                         name="scr")
                    # pad ragged slice columns with zero outer products so
                    # the reduction can run full-width
                    for s in range(NSL):
                        for col in range(td["ccount"][s], td["cmax"]):
                            nc.tensor.matmul(
                                td["ps_d"][0:P, s, col:col + 1],
                                etiles[(ti, 0)][:, 0:P], zero_c[:],
                                start=True, stop=True,
                                skip_group_check=True)
                ps_d = td["ps_d"]
                et = etiles.pop((ti, ci))
                first = (ci == 0)
                last = (ci == len(td["chunks"]) - 1)
                nc.tensor.matmul(
                    td["ps_o"][:, j0:], vsbs[c // NSL][:, c % NSL, :],
                    et[:, j0:], start=first, stop=last)
                # rowsums via stationary exp tiles: 1 moving row per slice;
                # each partial is its own committed single-instruction group
                for s in range(NSL):
                    lo = max(j0, s * P)
                    hi = (s + 1) * P
                    if lo >= hi:
                        continue
                    off = lo - s * P
                    col = td["colof"][(ci, s)]
                    if off > 0:
                        nc.vector.memset(ps_d[0:off, s, col:col + 1], 0.0)
                    nc.tensor.matmul(
                        ps_d[off:P, s, col:col + 1], et[:, lo:hi], ones_c[:],
                        start=True, stop=True, skip_group_check=True)

                def reduce_slice(s):
                    cs = td["ccount"][s]
                    nc.scalar.activation(
                        out=td["scr"][:, s, 0:cs], in_=ps_d[:, s, 0:cs],
                        func=AF.Copy, accum_out=td["dsum"][:, s:s + 1])

                if ti == last_ti:
                    # two grouped chains: early slices right before the last
                    # chunk, the final slice at the very end
                    max_ci = len(td["chunks"]) - 1
                    grp_a = [s for s in range(NSL)
                             if td["sl_last"][s] is not None
                             and td["sl_last"][s] < max_ci]
                    grp_b = [s for s in range(NSL)
                             if td["sl_last"][s] == max_ci]
                    if grp_a and ci == max(td["sl_last"][s] for s in grp_a):
                        for s in grp_a:
                            reduce_slice(s)
                        norm_slices(td, ti, min(grp_a), max(grp_a))
                    if grp_b and ci == max_ci:
                        for s in grp_b:
                            reduce_slice(s)
                        norm_slices(td, ti, min(grp_b), max(grp_b))
                elif last:
                    # reciprocal + broadcast DMAs fire now; the normalize
                    # multiply is deferred one tile so the bc DMA has a full
                    # tile of latency budget and never parks the DVE queue
                    cm = td["cmax"]
                    ds = rsp.tile([P, NSL, NCH], F32, tag="dt", name="ds")
                    nc.vector.tensor_copy(ds[:, :, 0:cm], ps_d[:, :, 0:cm])
                    m = cm
                    while m > 1:
                        h = m // 2
                        nc.vector.tensor_tensor(
                            out=ds[:, :, 0:h], in0=ds[:, :, 0:h],
                            in1=ds[:, :, m - h:m], op=allp.add)
                        m = m - h
                    rs = rsp.tile([P, NSL], F32, tag="rs", name="rs")
                    nc.vector.reciprocal(out=rs[:], in_=ds[:, :, 0])
                    nc.sync.dma_start(
                        out=rs_scr[ti].rearrange("o (s p) -> p (o s)", p=P),
                        in_=rs[:])
                    bc = tpool.tile([P, ST], F32, tag="bc", name="bc")
                    nc.sync.dma_start(
                        out=bc[:], in_=rs_scr[ti].partition_broadcast(P))
                    td["bc"] = bc

            def emit_norm(ti):
                td = tiles[ti]
                cols = bass.ts(td["t"], ST)
                hh = td["hh"]
                tmp = tpool.tile([P, ST], F32, tag="tmp", name="tmp")
                nc.vector.tensor_tensor(
                    out=tmp[:], in0=td["ps_o"][:], in1=td["bc"][:],
                    op=allp.mult)
                with nc.allow_low_precision(reason="fp8 codes"):
                    nc.vector.tensor_copy(oh[:, hh, cols], tmp[:])
                    nc.vector.tensor_tensor(
                        out=ol[:, hh, cols], in0=tmp[:],
                        in1=oh[:, hh, cols], op=allp.subtract)

            def outproj_tq(tq, tail=False):
                rows = bass.ts(tq, P)
                ysb = None
                for n in range(DIM // ST):
                    ncols = bass.ts(n, ST)
                    if tail:
                        # attention is drained: borrow the po ring so the
                        # tail out-proj rotates over 4 PSUM banks
                        ypool_n = py if n % 2 == 0 else po
                        ps_y = ypool_n.tile([P, ST], F32,
                                            tag="y" if n % 2 == 0 else "o",
                                            name="ps_y")
                    else:
                        ps_y = py.tile([P, ST], F32, tag="y", name="ps_y")
                    for pidx in range(HPC // 2):
                        hsl = slice(2 * pidx, 2 * pidx + 2)
                        nc.tensor.matmul(
                            ps_y[:], oh[:, hsl, rows],
                            woh_sb[:, hsl, ncols],
                            start=pidx == 0, stop=False, perf_mode=DR)
                        nc.tensor.matmul(
                            ps_y[:], ol[:, hsl, rows],
                            woh_sb[:, hsl, ncols],
                            start=False, stop=False, perf_mode=DR)
                        nc.tensor.matmul(
                            ps_y[:], oh[:, hsl, rows],
                            wol_sb[:, hsl, ncols],
                            start=False, stop=pidx == HPC // 2 - 1,
                            perf_mode=DR)
                    if ysb is None:
                        ysb = ypool.tile([P, 2, ST], F16, tag="ys",
                                         name="ysb")
                    half = n % 2
                    with nc.allow_low_precision(reason="f16 partials"):
                        if half == 0 or not tail:
                            nc.vector.tensor_scalar_mul(
                                ysb[:, half], ps_y[:], y_desc)
                        else:
                            nc.scalar.activation(
                                out=ysb[:, half], in_=ps_y[:],
                                func=AF.Copy, scale=y_desc)
                    if tail and tq % NSL == NSL - 1:
                        yq = nc.sync if n % 2 == 1 else nc.scalar
                        yq.dma_start(
                            out=y[tq * P:(tq + 1) * P, n * ST:(n + 1) * ST],
                            in_=ysb[:, half])
                        if half == 1:
                            ysb = None
                    elif half == 1:
                        if tail:
                            yq = nc.sync if n % 4 == 1 else nc.scalar
                        else:
                            yq = nc.gpsimd
                        yq.dma_start(
                            out=y[tq * P:(tq + 1) * P,
                                  (n - 1) * ST:(n + 1) * ST],
                            in_=ysb[:])
                        ysb = None

            def outproj(t):
                # y rows of s-tile t: all 4 heads' oh/ol just finished
                for tq in range(t * NSL, (t + 1) * NSL):
                    outproj_tq(tq)

            np_ = 0
            pending = []

            def drain_pending():
                while pending:
                    pti = pending.pop(0)
                    emit_norm(pti)
                    if tiles[pti]["hh"] == HPC - 1:
                        outproj(tiles[pti]["t"])

            for i in range(len(flat)):
                while np_ < min(i + 1 + LOOKAHEAD, len(flat)):
                    produce(*flat[np_])
                    np_ += 1
                ti, ci = flat[i]
                td = tiles[ti]
                final = ci == len(td["chunks"]) - 1
                if final and pending and (len(pending) > 1
                                          or td["t"] == NST - 1):
                    # emit the previous tile's norm BEFORE this tile's last
                    # chunk: its inputs are long ready, so it must not queue
                    # behind this tile's parked reciprocal on DVE
                    pti = pending.pop(0)
                    emit_norm(pti)
                    if tiles[pti]["hh"] == HPC - 1:
                        outproj(tiles[pti]["t"])
                consume(*flat[i])
                if final:
                    if td["t"] == NST - 1 and ti != last_ti:
                        drain_pending()
                        emit_norm(ti)
                    elif ti == last_ti:
                        pass  # per-slice path handled in consume()
                    else:
                        pending.append(ti)
            drain_pending()
            # tail out-proj for the final s-tile, row-block order so the
            # per-slice norm chains are already satisfied front-to-back
            t_last = tiles[last_ti]["t"]
            for s in range(NSL):
                outproj_tq(t_last * NSL + s, tail=True)
    return nc


# ------------------------------------------------------------- host driver
def _classify_mask(mask2d):
    """Classify [128, 512] blocks of mask^T.

    Returns (block_kind, mixed_vals, exact_binary): block_kind[(c, t)] is
    ('allow', 0) | 'skip' | ('mixed', idx, j0, j0a, j1a) where j0 is the PE
    compute-window start (leading fully-masked columns, capped at 256) and
    [j0a, j1a) the mask window; mixed_vals is [n_mixed, 128, 512] of mask^T
    values pre-scaled by sqrt(DH), clipped to stay finite; exact_binary is
    True when every mixed value is 0 or <= -1e30."""
    mT = mask2d.T  # [sk, sq]
    block_kind = {}
    mixed = []
    exact_binary = True
    for c in range(NCH):
        for t in range(NST):
            blk = mT[c * P:(c + 1) * P, t * ST:(t + 1) * ST]
            if not blk.any():
                block_kind[(c, t)] = ("allow", 0)
            elif (blk <= NEG_THRESH).all():
                block_kind[(c, t)] = "skip"
            else:
                allmasked = (blk <= NEG_THRESH).all(axis=0)
                j0 = 0
                while j0 < ST and allmasked[j0]:
                    j0 += 1
                j0 = min(j0, ST - 256, 256)
                j0 = max(j0, 0)
                nz = np.flatnonzero(blk.any(axis=0))
                j0a, j1a = int(nz[0]), int(nz[-1]) + 1
                j0a = max(j0a, j0)
                if j1a <= j0a:
                    block_kind[(c, t)] = ("allow", j0)
                    continue
                block_kind[(c, t)] = ("mixed", len(mixed), j0, j0a, j1a)
                if not ((blk == 0) | (blk <= NEG_THRESH)).all():
                    exact_binary = False
                scaled = np.clip(
                    blk[:, j0a:j1a].astype(np.float64) * math.sqrt(DH),
                    -3e38, 3e38).astype(np.float32)
                mixed.append(scaled)
    if mixed:
        wmax = max(m.shape[1] for m in mixed)
        mv = np.zeros((len(mixed), P, wmax), np.float32)
        for i, m in enumerate(mixed):
            mv[i, :, :m.shape[1]] = m
    else:
        mv = None
    return block_kind, mv, exact_binary


def _rope_tables(position_ids, desc):
    """packed [DH, NST, 2, ST]: [..., 0, :] = cos, [..., 1, :] = signed sin,
    both pre-scaled by the projection-code descale."""
    pos = position_ids.reshape(-1).astype(np.float64)  # [S]
    inv = 1.0 / (THETA ** (np.arange(0, DH, 2, dtype=np.float64) / DH))
    fr = pos[None, :] * inv[:, None]          # [64, S]
    cosT = np.empty((DH, S), np.float32)
    sinTs = np.empty((DH, S), np.float32)
    cosT[0:64] = np.cos(fr) * desc
    cosT[64:128] = np.cos(fr) * desc
    sinTs[0:64] = -np.sin(fr) * desc
    sinTs[64:128] = np.sin(fr) * desc
    cstab = np.stack([cosT.reshape(DH, NST, ST),
                      sinTs.reshape(DH, NST, ST)], axis=2)
    return np.ascontiguousarray(cstab)


def _prep_inputs(x, mask, position_ids, Wq, Wk, Wv, Wo):
    x = np.asarray(x, np.float32)
    Wq = np.asarray(Wq, np.float32)
    Wk = np.asarray(Wk, np.float32)
    Wv = np.asarray(Wv, np.float32)
    Wo = np.asarray(Wo, np.float32)
    block_kind, mixed_vals, exact_binary = _classify_mask(
        np.asarray(mask)[0, 0])
    if mixed_vals is not None and exact_binary:
        # additive mask values ride a PE identity-matmul in bf16
        mixed_vals = mixed_vals.astype(ml_dtypes.bfloat16)

    # global power-2 scales shared by all cores (compiled constants)
    ax = int(np.floor(np.log2(120.0 / max(float(np.abs(x).max()), 1e-30))))
    sqk = 2.0 ** int(np.floor(np.log2(
        120.0 / max(float(np.abs(Wq).max()), float(np.abs(Wk).max()), 1e-30))))
    bqk = int(np.log2(sqk))
    bv = int(np.floor(np.log2(120.0 / max(float(np.abs(Wv).max()), 1e-30))))
    bo = int(np.floor(np.log2(120.0 / max(float(np.abs(Wo).max()), 1e-30))))
    scales = {"ax": ax, "bqk": bqk, "bv": bv, "bo": bo}

    def enc_fixed(t, e):
        ts = np.asarray(t, np.float32) * (2.0 ** e)
        hi = ts.astype(E4NP)
        lo = (ts - hi.astype(np.float32)).astype(E4NP)
        return np.ascontiguousarray(hi), np.ascontiguousarray(lo)

    def pack_pairs(w):
        # [DIM, M] -> [P, NPAIR, 2, M] pair-interleaved SBUF layout
        return np.ascontiguousarray(
            w.reshape(NPAIR, 2, P, -1).transpose(2, 0, 1, 3))

    desc = 2.0 ** (-(ax + bqk))
    cstab = _rope_tables(np.asarray(position_ids), desc)
    onescol = np.full((P, 1), 2.0 ** (-OC), ml_dtypes.bfloat16)
    zerocol = np.zeros((P, 1), ml_dtypes.bfloat16)
    onesrow = np.ones((1, P), np.float32)
    ident = np.eye(P, dtype=np.float32)
    identb = np.eye(P, dtype=ml_dtypes.bfloat16)

    in_maps = []
    for core in range(NCORE):
        b, g = divmod(core, KVH)
        xbh, xbl = enc_fixed(x[b].T, ax)
        # v uses its own scale; encode v/k separately then pack fp8 codes
        vh_, vl_ = enc_fixed(Wv[:, g * DH:(g + 1) * DH], bv)
        kh_, kl_ = enc_fixed(Wk[:, g * DH:(g + 1) * DH], bqk)
        vkh = np.ascontiguousarray(
            pack_pairs(np.concatenate([vh_, kh_], axis=1)))
        vkl = np.ascontiguousarray(
            pack_pairs(np.concatenate([vl_, kl_], axis=1)))
        qh_, ql_ = enc_fixed(Wq[:, g * HPC * DH:(g + 1) * HPC * DH], bqk)
        qhl = np.ascontiguousarray(np.stack(
            [pack_pairs(qh_), pack_pairs(ql_)], axis=1))
        oh_, ol_ = enc_fixed(Wo[g * HPC * DH:(g + 1) * HPC * DH, :], bo)
        m = {
            "xh": xbh, "xl": xbl,
            "wvkh": vkh, "wvkl": vkl, "wqhl": qhl,
            "woh": oh_, "wol": ol_,
            "cstab": cstab,
            "onescol": onescol, "zerocol": zerocol,
            "onesrow": onesrow, "ident": ident,
            "identb": identb,
        }
        if mixed_vals is not None:
            m["mmask"] = mixed_vals
        in_maps.append(m)
    return (block_kind, scales, exact_binary), in_maps


def kernel(x, mask, position_ids, Wq, Wk, Wv, Wo):
    (block_kind, scales, exact_binary), in_maps = _prep_inputs(
        x, mask, position_ids, Wq, Wk, Wv, Wo)
    nc = build_program(block_kind, scales, mask_exact_binary=exact_binary)
    res = run_bass_kernel_spmd(nc, in_maps, core_ids=list(range(NCORE)))
    out = np.zeros((B, S, DIM), np.float32)
    for core in range(NCORE):
        b = core // KVH
        out[b] += res.results[core]["y"].astype(np.float32)
    return out
